# revision 26
# baseline (speedup 1.0000x reference)
# Trainium2 Bass kernel for nn_EquivariantCorrectionHead — v3.
# All-eigen sss (ACT squares PSUM), Pool tree-L1 + DVE finish reduces,
# XBAR transposes for tp2, 4-group macro tiles.

import sys
from contextlib import ExitStack

import numpy as np

if "/opt/trn_rl_repo" not in sys.path:
    sys.path.insert(0, "/opt/trn_rl_repo")

import concourse.bass as bass
import concourse.mybir as mybir
import concourse.tile as tile
from concourse import bacc, masks
from concourse.bass_utils import run_bass_kernel_spmd

B, NS, H = 131072, 64, 32
NCORES = 8
BPC = B // NCORES
P = 128
G = 4
MS = P * G
NMACRO = BPC // MS

PW1_0 = float((NS * NS + 25.0) ** -0.5)
PW1_2 = float((5.0 / (10.0 * NS + 25.0)) ** 0.5)
PW2_2 = float((5.0 / (3.0 * H * H)) ** 0.5)
INV_S5 = float(5.0 ** -0.5)

F32 = mybir.dt.float32
F16 = mybir.dt.float16
AX = mybir.AxisListType
OP = mybir.AluOpType
AF = mybir.ActivationFunctionType

CE = 64 + 32 * 64          # 2112: id + 32 eigen w's
CAE = 160 + 125
NCOL = CE + CAE            # 2397


def _wigner3j_222():
    s2, s6 = np.sqrt(2.0), np.sqrt(6.0)
    M = np.zeros((5, 3, 3))
    M[0] = np.array([[0.0, 1, 0], [1, 0, 0], [0, 0, 0]]) / s2
    M[1] = np.array([[0.0, 0, 0], [0, 0, 1], [0, 1, 0]]) / s2
    M[2] = np.diag([-1.0, -1, 2]) / s6
    M[3] = np.array([[0.0, 0, 1], [0, 0, 0], [1, 0, 0]]) / s2
    M[4] = np.diag([1.0, -1, 0]) / s2
    C = np.einsum("aij,bjk,cki->abc", M, M, M)
    C = 0.5 * (C + C.transpose(1, 0, 2))
    return C / np.linalg.norm(C)


def prep_weights(w1_sss, w1_stt, w1_tst, w1_tts, w1_ttt, w2_stt, w2_tst, w2_ttt):
    C = _wigner3j_222()
    w1_sss = np.asarray(w1_sss, np.float64)
    w1_stt = np.asarray(w1_stt, np.float64)
    w1_tst = np.asarray(w1_tst, np.float64)
    w1_tts = np.asarray(w1_tts, np.float64)
    w1_ttt = np.asarray(w1_ttt, np.float64)
    w2_stt = np.asarray(w2_stt, np.float64)[:, :, 0]
    w2_tst = np.asarray(w2_tst, np.float64)[:, :, 0]
    w2_ttt = np.asarray(w2_ttt, np.float64)[:, :, 0]

    Wall = np.zeros((89, NCOL))
    cvec = np.zeros(32)
    Wall[0:64, 0:64] = np.eye(64)
    for w in range(32):
        A = PW1_0 * w1_sss[:, :, w]
        S = 0.5 * (A + A.T)
        lam, Q = np.linalg.eigh(S)
        c = max(0.0, -lam.min()) + 1e-6
        Wall[0:64, 64 + w * 64: 64 + (w + 1) * 64] = Q * np.sqrt(lam + c)
        cvec[w] = c
    wb_ad = (PW1_2 * INV_S5) * (
        np.transpose(w1_stt, (0, 2, 1)) + np.transpose(w1_tst, (1, 2, 0))
    ).reshape(NS, H * 5)
    Wall[0:64, CE: CE + 160] = wb_ad
    Cbig = np.zeros((25, 125))
    for u in range(5):
        for j in range(5):
            for k in range(5):
                Cbig[u * 5: u * 5 + 5, k * 25 + u * 5 + j] = C[:, j, k]
    Wall[64:89, CE + 160: NCOL] = Cbig

    Wtts = (PW1_0 * INV_S5) * w1_tts.reshape(25, H)
    wttt = PW1_2 * w1_ttt.reshape(25, H)
    WtttA = np.zeros((125, 128))
    WtttB = np.zeros((125, 32))
    for k in range(4):
        WtttA[k * 25: k * 25 + 25, k * 32: k * 32 + 32] = wttt
    WtttB[100:125, :] = wttt
    w2t = PW2_2 * w2_ttt
    W2A = np.zeros((128, 128))
    for i in range(4):
        W2A[i * 32: i * 32 + 32, i * 32: i * 32 + 32] = w2t
    W2B = w2t.copy()
    M2 = (PW2_2 * INV_S5) * (w2_stt + w2_tst.T)
    crepC = np.broadcast_to(np.transpose(C, (2, 0, 1)).reshape(1, 125), (P, 125)).copy()
    crep_cw = np.broadcast_to(cvec.reshape(1, 32), (P, 32)).copy()

    f16 = lambda x: np.ascontiguousarray(x, np.float16)
    return {
        "Wall": f16(Wall), "Wtts": f16(Wtts), "WtttA": f16(WtttA),
        "WtttB": f16(WtttB), "W2A": f16(W2A), "W2B": f16(W2B),
        "M2": f16(M2), "CrepC": f16(crepC), "CrepW": f16(crep_cw),
    }


WEIGHT_SHAPES = {
    "Wall": (89, NCOL), "Wtts": (25, H), "WtttA": (125, 128),
    "WtttB": (125, 32), "W2A": (128, 128), "W2B": (32, 32),
    "M2": (32, 32), "CrepC": (P, 125), "CrepW": (P, 32),
}

WS_M = 0
WS_G = 128


def _tile_body(ctx: ExitStack, tc: tile.TileContext, io, n_macro: int):
    nc = tc.nc
    s_d, t_d, stT_d, out_d, wd = io["s"], io["t"], io["stT"], io["out"], io["w"]
    tkr_d = io["tkr"]

    const = ctx.enter_context(tc.tile_pool(name="const", bufs=1))
    W = {}
    for name, shp in WEIGHT_SHAPES.items():
        W[name] = const.tile(list(shp), F16, tag=name, name=f"W_{name}")
        nc.sync.dma_start(W[name][:], wd[name])
    ident32 = const.tile([128, 128], F32, tag="ident32")
    masks.make_identity(nc, ident32[:])

    io_pool = ctx.enter_context(tc.tile_pool(name="io", bufs=2))
    zq = ctx.enter_context(tc.tile_pool(name="zq", bufs=2))
    sb = ctx.enter_context(tc.tile_pool(name="sb", bufs=2))
    fm = ctx.enter_context(tc.tile_pool(name="fm", bufs=2))
    zps = ctx.enter_context(tc.tile_pool(name="zps", bufs=1, space="PSUM"))
    tpsA = ctx.enter_context(tc.tile_pool(name="tpsA", bufs=2, space="PSUM"))
    tpsB = ctx.enter_context(tc.tile_pool(name="tpsB", bufs=1, space="PSUM"))

    ctx.enter_context(nc.allow_low_precision("fp16 intermediates fit the 2e-2 budget"))

    for im in range(n_macro):
        r0 = im * MS
        st4 = io_pool.tile([P, G, 96], F16, tag="st4")
        nc.sync.dma_start(st4[:, :, 0:64], s_d[r0: r0 + MS, :].rearrange("(g p) u -> p g u", g=G))
        nc.sync.dma_start(st4[:, :, 64:89], t_d[r0: r0 + MS, :].rearrange("(g p) u -> p g u", g=G))
        stT4 = io_pool.tile([89, MS], F16, tag="stT4")
        nc.sync.dma_start(stT4[:], stT_d[:, r0: r0 + MS])

        zsqE = zq.tile([P, G, CE], F16, tag="zsqE")
        ae4 = sb.tile([P, G, 285], F16, tag="ae4")

        for g in range(G):
            lhs = stT4[:, g * P: (g + 1) * P]
            zp = zps.tile([P, NCOL], F32, tag="zp")
            for c0 in range(0, 2048, 512):
                nc.tensor.matmul(zp[:, c0: c0 + 512], lhs, W["Wall"][:, c0: c0 + 512], start=True, stop=True)
            nc.tensor.matmul(zp[:, 2048:NCOL], lhs, W["Wall"][:, 2048:NCOL], start=True, stop=True)
            # ACT squares everything (id + 32 eigen w's)
            nc.scalar.activation(zsqE[:, g, 0:1024], zp[:, 0:1024], AF.Square)
            nc.scalar.activation(zsqE[:, g, 1024:CE], zp[:, 1024:CE], AF.Square)
            nc.scalar.activation(ae4[:, g, :], zp[:, CE:NCOL], AF.Copy)

        # ---- h1 ----
        tmp_e = sb.tile([P, G, 33], F32, tag="tmp_e")
        zhE = zq.tile([P, G, 33, 32], F16, tag="zhE")
        zE4 = zsqE[:].rearrange("p g (w h v) -> p (g w) h v", h=2, v=32)
        nc.gpsimd.tensor_tensor(
            zhE[:].rearrange("p g w v -> p (g w) v"), zE4[:, :, 0, :], zE4[:, :, 1, :], OP.add)
        nc.vector.tensor_reduce(
            tmp_e[:], zhE[:].rearrange("p g w v -> p (g w) v"), axis=AX.X, op=OP.add)

        ws = sb.tile([P, G, 256], F16, tag="ws")
        h1f = sb.tile([P, G, 32], F32, tag="h1f")
        tmp2 = sb.tile([P, G, 32], F32, tag="tmp2")
        s2b = tmp_e[:, :, 0:1].to_broadcast((P, G, 32))
        cwb = W["CrepW"][:].unsqueeze(1).to_broadcast((P, G, 32))
        nc.vector.tensor_tensor(tmp2[:], s2b, cwb, OP.mult)
        nc.vector.tensor_tensor(h1f[:], tmp_e[:, :, 1:33], tmp2[:], OP.subtract)

        # ---- small bilinears ----
        tkr = sb.tile([P, G, 25], F16, tag="tkr")
        nc.sync.dma_start(tkr[:], tkr_d[r0: r0 + MS, :].rearrange("(g p) u -> p g u", g=G))
        q24 = zq.tile([P, G, 800], F16, tag="q24")
        for g in range(G):
            a4v = ae4[:, g, 0:160].rearrange("p (w r) -> p w r", w=H).unsqueeze(1).to_broadcast((P, 5, H, 5))
            t_kr = tkr[:, g, :].rearrange("p (k r) -> p k r", k=5).unsqueeze(2).to_broadcast((P, 5, H, 5))
            eng = nc.gpsimd if g < 3 else nc.vector
            eng.tensor_tensor(q24[:, g, :].rearrange("p (k w r) -> p k w r", k=5, w=H), a4v, t_kr, OP.mult)
        ht1f = sb.tile([P, G, 160], F32, tag="ht1f")
        nc.vector.tensor_reduce(
            ht1f[:],
            q24[:].rearrange("p g (c r) -> p (g c) r", r=5), axis=AX.X, op=OP.add)

        qg4 = zq.tile([P, G, 125], F16, tag="qg4")
        for g in range(G):
            t_ui = st4[:, g, 64:89].rearrange("p (u i) -> p u i", u=5).unsqueeze(2).to_broadcast((P, 5, 5, 5))
            t_vi = st4[:, g, 64:89].rearrange("p (v i) -> p v i", v=5).unsqueeze(1).to_broadcast((P, 5, 5, 5))
            eng = nc.gpsimd if g < 2 else nc.vector
            eng.tensor_tensor(qg4[:, g, :].rearrange("p (u v i) -> p u v i", u=5, v=5), t_ui, t_vi, OP.mult)
        nc.vector.tensor_reduce(
            ws[:, :, WS_G: WS_G + 25],
            qg4[:].rearrange("p g (c i) -> p (g c) i", i=5), axis=AX.X, op=OP.add)

        q74 = zq.tile([P, G, 625], F16, tag="q74")
        for g in range(G):
            E3 = ae4[:, g, 160:285].rearrange("p (c j) -> p c j", j=5).unsqueeze(2).to_broadcast((P, 25, 5, 5))
            t_vj = st4[:, g, 64:89].rearrange("p (v j) -> p v j", v=5).unsqueeze(1).to_broadcast((P, 25, 5, 5))
            eng = nc.gpsimd if g < 3 else nc.vector
            eng.tensor_tensor(q74[:, g, :].rearrange("p (c v j) -> p c v j", c=25, v=5), E3, t_vj, OP.mult)
        nc.vector.tensor_reduce(
            ws[:, :, WS_M: WS_M + 125],
            q74[:].rearrange("p g (c j) -> p (g c) j", j=5), axis=AX.X, op=OP.add)

        # ---- XBAR forward ----
        xb = fm.tile([P, 2 * G, P], F16, tag="xb")
        nc.sync.dma_start_transpose(xb[:], ws[:].rearrange("p g c -> p (g c)"))
        mt_rhs = xb[0:125, :, :].rearrange("f (g b) s -> f g b s", b=2)[:, :, 0, :]
        gt_rhs = xb[0:25, :, :].rearrange("f (g b) s -> f g b s", b=2)[:, :, 1, :]

        # ---- tp2 feature-major; psum evicted by ACT, added by Pool ----
        htA_ps = tpsA.tile([P, MS], F32, tag="tpA")
        nc.tensor.matmul(htA_ps[:], W["WtttA"][:], mt_rhs, start=True, stop=False)
        for g in range(G):
            nc.tensor.matmul(htA_ps[:, g * P:(g + 1) * P], ht1f[:, g, 0:128], ident32[:],
                             is_transpose=True, start=False, stop=(g == G - 1))
        htB_ps = tpsB.tile([32, MS], F32, tag="tpB")
        nc.tensor.matmul(htB_ps[:], W["WtttB"][:], mt_rhs, start=True, stop=False)
        for g in range(G):
            nc.tensor.matmul(htB_ps[:, g * P:(g + 1) * P], ht1f[:, g, 128:160], ident32[:],
                             is_transpose=True, start=False, stop=(g == G - 1))
        htA = fm.tile([P, MS], F16, tag="htAf")
        nc.scalar.copy(htA[:], htA_ps[:])
        pk96 = fm.tile([96, MS], F16, tag="pk96")
        nc.scalar.copy(pk96[0:32, :], htB_ps[:])

        hs_ps = tpsB.tile([32, MS], F32, tag="tpB")
        nc.tensor.matmul(hs_ps[:], W["Wtts"][:], gt_rhs, start=True, stop=False)
        for g in range(G):
            nc.tensor.matmul(hs_ps[:, g * P:(g + 1) * P], h1f[:, g, :], ident32[:],
                             is_transpose=True, start=False, stop=(g == G - 1))
        hs_fm = fm.tile([32, MS], F16, tag="hsf")
        nc.scalar.copy(hs_fm[:], hs_ps[:])

        al_ps = tpsB.tile([32, MS], F32, tag="tpB")
        nc.tensor.matmul(al_ps[:], W["M2"][:], hs_fm[:], start=True, stop=True)
        nc.scalar.activation(pk96[64:96, :], al_ps[:], AF.Copy)

        g2A_ps = tpsA.tile([P, MS], F32, tag="tpA")
        nc.tensor.matmul(g2A_ps[:], W["W2A"][:], htA[:], start=True, stop=True)
        g2B_ps = tpsB.tile([32, MS], F32, tag="tpB")
        nc.tensor.matmul(g2B_ps[:], W["W2B"][:], pk96[0:32, :], start=True, stop=True)
        g2A = fm.tile([P, MS], F16, tag="g2Af")
        nc.scalar.activation(g2A[:], g2A_ps[:], AF.Copy)
        nc.scalar.activation(pk96[32:64, :], g2B_ps[:], AF.Copy)

        # ---- XBAR back ----
        xhA = fm.tile([P, G, P], F16, tag="xhA")
        nc.sync.dma_start_transpose(xhA[:], htA[:])
        xgA = fm.tile([P, G, P], F16, tag="xgA")
        nc.sync.dma_start_transpose(xgA[:], g2A[:])
        xp = fm.tile([P, G, 96], F16, tag="xp")
        nc.sync.dma_start_transpose(xp[:], pk96[:])

        htb4 = sb.tile([P, G, 160], F16, tag="htb4")
        nc.gpsimd.tensor_copy(htb4[:, :, 0:128], xhA[:])
        nc.gpsimd.tensor_copy(htb4[:, :, 128:160], xp[:, :, 0:32])
        g2b4 = sb.tile([P, G, 160], F16, tag="g2b4")
        nc.gpsimd.tensor_copy(g2b4[:, :, 0:128], xgA[:])
        nc.gpsimd.tensor_copy(g2b4[:, :, 128:160], xp[:, :, 32:64])

        qq4 = zq.tile([P, G, 800], F16, tag="qq4")
        for g in range(G):
            g2v = g2b4[:, g, :].rearrange("p (i v) -> p i v", i=5).unsqueeze(2).to_broadcast((P, 5, 5, 32))
            htv = htb4[:, g, :].rearrange("p (j v) -> p j v", j=5).unsqueeze(1).to_broadcast((P, 5, 5, 32))
            eng = nc.gpsimd if g < 2 else nc.vector
            eng.tensor_tensor(qq4[:, g, :].rearrange("p (i j v) -> p i j v", i=5, j=5), g2v, htv, OP.mult)
        Q4 = sb.tile([P, G, 25], F16, tag="Q4")
        qqh = zq.tile([P, G, 25, 16], F16, tag="qqh")
        qv = qq4[:].rearrange("p g (c v) -> p (g c) v", v=32)
        nc.gpsimd.tensor_tensor(
            qqh[:].rearrange("p g c v -> p (g c) v"), qv[:, :, 0:16], qv[:, :, 16:32], OP.add)
        nc.vector.tensor_reduce(
            Q4[:], qqh[:].rearrange("p g c v -> p (g c) v"), axis=AX.X, op=OP.add)

        q10 = zq.tile([P, G, 125], F16, tag="q10")
        for g in range(G):
            Qb = Q4[:, g, :].unsqueeze(1).to_broadcast((P, 5, 25))
            cv = W["CrepC"][:].rearrange("p (k c) -> p k c", k=5)
            eng = nc.gpsimd if g < 2 else nc.vector
            eng.tensor_tensor(q10[:, g, :].rearrange("p (k c) -> p k c", k=5), Qb, cv, OP.mult)
        q12 = zq.tile([P, G, 160], F16, tag="q12")
        for g in range(G):
            alb = xp[:, g, 64:96].unsqueeze(1).to_broadcast((P, 5, 32))
            htk = htb4[:, g, :].rearrange("p (k v) -> p k v", k=5)
            eng = nc.gpsimd if g < 2 else nc.vector
            eng.tensor_tensor(q12[:, g, :].rearrange("p (k v) -> p k v", k=5), alb, htk, OP.mult)
        o14 = sb.tile([P, G, 5], F32, tag="o14")
        nc.vector.tensor_reduce(
            o14[:], q10[:].rearrange("p g (c j) -> p (g c) j", j=25), axis=AX.X, op=OP.add)
        o24 = sb.tile([P, G, 5], F32, tag="o24")
        nc.vector.tensor_reduce(
            o24[:], q12[:].rearrange("p g (c v) -> p (g c) v", v=32), axis=AX.X, op=OP.add)
        out4 = io_pool.tile([P, G, 5], F32, tag="out4")
        nc.vector.tensor_add(out4[:], o14[:], o24[:])
        nc.sync.dma_start(out_d[r0: r0 + MS, :].rearrange("(g p) c -> p g c", g=G), out4[:])


def build_program(n_macro=NMACRO):
    nc = bacc.Bacc("TRN2", target_bir_lowering=False, debug=False,
                   enable_asserts=False, num_devices=NCORES)
    rows = n_macro * MS
    io = {
        "s": nc.dram_tensor("s", [rows, NS], F16, kind="ExternalInput").ap(),
        "t": nc.dram_tensor("t", [rows, 25], F16, kind="ExternalInput").ap(),
        "stT": nc.dram_tensor("stT", [89, rows], F16, kind="ExternalInput").ap(),
        "tkr": nc.dram_tensor("tkr", [rows, 25], F16, kind="ExternalInput").ap(),
        "out": nc.dram_tensor("out", [rows, 5], F32, kind="ExternalOutput").ap(),
        "w": {name: nc.dram_tensor(name, list(shp), F16, kind="ExternalInput").ap()
              for name, shp in WEIGHT_SHAPES.items()},
    }
    with tile.TileContext(nc) as tc:
        with ExitStack() as ctx:
            _tile_body(ctx, tc, io, n_macro)
    nc.compile()
    return nc


def make_in_maps(
    scalars, kernel_t2_sum, mc_t2, coulomb_t2, bs_t2, mopac_coulomb_t2,
    w1_sss, w1_stt, w1_tst, w1_tts, w1_ttt, w2_stt, w2_tst, w2_ttt,
):
    wmap = prep_weights(w1_sss, w1_stt, w1_tst, w1_tts, w1_ttt, w2_stt, w2_tst, w2_ttt)
    s = np.ascontiguousarray(np.asarray(scalars, np.float16))
    t = np.stack(
        [np.asarray(kernel_t2_sum, np.float32), np.asarray(mc_t2, np.float32),
         np.asarray(coulomb_t2, np.float32), np.asarray(bs_t2, np.float32),
         np.asarray(mopac_coulomb_t2, np.float32)], axis=1,
    ).reshape(B, 25).astype(np.float16)
    in_maps = []
    for c in range(NCORES):
        sh = s[c * BPC: (c + 1) * BPC]
        th = t[c * BPC: (c + 1) * BPC]
        stT = np.concatenate([sh.T, th.T], axis=0)
        tk = np.ascontiguousarray(th.reshape(-1, 5, 5).transpose(0, 2, 1).reshape(-1, 25))
        m = {"s": sh, "t": np.ascontiguousarray(th), "stT": np.ascontiguousarray(stT), "tkr": tk}
        m.update(wmap)
        in_maps.append(m)
    return in_maps


_CACHED_NC = None


def kernel(
    scalars, kernel_t2_sum, mc_t2, coulomb_t2, bs_t2, mopac_coulomb_t2,
    w1_sss, w1_stt, w1_tst, w1_tts, w1_ttt, w2_stt, w2_tst, w2_ttt,
):
    global _CACHED_NC
    if _CACHED_NC is None:
        _CACHED_NC = build_program(NMACRO)
    nc = _CACHED_NC
    in_maps = make_in_maps(
        scalars, kernel_t2_sum, mc_t2, coulomb_t2, bs_t2, mopac_coulomb_t2,
        w1_sss, w1_stt, w1_tst, w1_tts, w1_ttt, w2_stt, w2_tst, w2_ttt,
    )
    res = run_bass_kernel_spmd(nc, in_maps, list(range(NCORES)))
    out = np.concatenate([res.results[c]["out"] for c in range(NCORES)], axis=0)
    return out.astype(np.float32)


# revision 27
# speedup vs baseline: 1.1860x; 1.1860x over previous
# Trainium2 Bass kernel for nn_EquivariantCorrectionHead — v3.
# All-eigen sss (ACT squares PSUM), Pool tree-L1 + DVE finish reduces,
# XBAR transposes for tp2, 4-group macro tiles.

import sys
from contextlib import ExitStack

import numpy as np

if "/opt/trn_rl_repo" not in sys.path:
    sys.path.insert(0, "/opt/trn_rl_repo")

import concourse.bass as bass
import concourse.mybir as mybir
import concourse.tile as tile
from concourse import bacc, masks
from concourse.bass_utils import run_bass_kernel_spmd

B, NS, H = 131072, 64, 32
NCORES = 8
BPC = B // NCORES
P = 128
G = 4
MS = P * G
NMACRO = BPC // MS

PW1_0 = float((NS * NS + 25.0) ** -0.5)
PW1_2 = float((5.0 / (10.0 * NS + 25.0)) ** 0.5)
PW2_2 = float((5.0 / (3.0 * H * H)) ** 0.5)
INV_S5 = float(5.0 ** -0.5)

F32 = mybir.dt.float32
F16 = mybir.dt.float16
AX = mybir.AxisListType
OP = mybir.AluOpType
AF = mybir.ActivationFunctionType

CE = 64 + 32 * 64          # 2112: id + 32 eigen w's
CAE = 160 + 125
NCOL = CE + CAE            # 2397


def _wigner3j_222():
    s2, s6 = np.sqrt(2.0), np.sqrt(6.0)
    M = np.zeros((5, 3, 3))
    M[0] = np.array([[0.0, 1, 0], [1, 0, 0], [0, 0, 0]]) / s2
    M[1] = np.array([[0.0, 0, 0], [0, 0, 1], [0, 1, 0]]) / s2
    M[2] = np.diag([-1.0, -1, 2]) / s6
    M[3] = np.array([[0.0, 0, 1], [0, 0, 0], [1, 0, 0]]) / s2
    M[4] = np.diag([1.0, -1, 0]) / s2
    C = np.einsum("aij,bjk,cki->abc", M, M, M)
    C = 0.5 * (C + C.transpose(1, 0, 2))
    return C / np.linalg.norm(C)


def prep_weights(w1_sss, w1_stt, w1_tst, w1_tts, w1_ttt, w2_stt, w2_tst, w2_ttt):
    C = _wigner3j_222()
    w1_sss = np.asarray(w1_sss, np.float64)
    w1_stt = np.asarray(w1_stt, np.float64)
    w1_tst = np.asarray(w1_tst, np.float64)
    w1_tts = np.asarray(w1_tts, np.float64)
    w1_ttt = np.asarray(w1_ttt, np.float64)
    w2_stt = np.asarray(w2_stt, np.float64)[:, :, 0]
    w2_tst = np.asarray(w2_tst, np.float64)[:, :, 0]
    w2_ttt = np.asarray(w2_ttt, np.float64)[:, :, 0]

    Wall = np.zeros((89, NCOL))
    cvec = np.zeros(32)
    Wall[0:64, 0:64] = np.eye(64)
    for w in range(32):
        A = PW1_0 * w1_sss[:, :, w]
        S = 0.5 * (A + A.T)
        lam, Q = np.linalg.eigh(S)
        c = max(0.0, -lam.min()) + 1e-6
        Wall[0:64, 64 + w * 64: 64 + (w + 1) * 64] = Q * np.sqrt(lam + c)
        cvec[w] = c
    wb_ad = (PW1_2 * INV_S5) * (
        np.transpose(w1_stt, (0, 2, 1)) + np.transpose(w1_tst, (1, 2, 0))
    ).reshape(NS, H * 5)
    Wall[0:64, CE: CE + 160] = wb_ad
    Cbig = np.zeros((25, 125))
    for u in range(5):
        for j in range(5):
            for k in range(5):
                Cbig[u * 5: u * 5 + 5, k * 25 + u * 5 + j] = C[:, j, k]
    Wall[64:89, CE + 160: NCOL] = Cbig

    Wtts = (PW1_0 * INV_S5) * w1_tts.reshape(25, H)
    wttt = PW1_2 * w1_ttt.reshape(25, H)
    WtttA = np.zeros((125, 128))
    WtttB = np.zeros((125, 32))
    for k in range(4):
        WtttA[k * 25: k * 25 + 25, k * 32: k * 32 + 32] = wttt
    WtttB[100:125, :] = wttt
    w2t = PW2_2 * w2_ttt
    W2A = np.zeros((128, 128))
    for i in range(4):
        W2A[i * 32: i * 32 + 32, i * 32: i * 32 + 32] = w2t
    W2B = w2t.copy()
    M2 = (PW2_2 * INV_S5) * (w2_stt + w2_tst.T)
    crepC = np.broadcast_to(np.transpose(C, (2, 0, 1)).reshape(1, 125), (P, 125)).copy()
    crep_cw = np.broadcast_to(cvec.reshape(1, 32), (P, 32)).copy()

    f16 = lambda x: np.ascontiguousarray(x, np.float16)
    return {
        "Wall": f16(Wall), "Wtts": f16(Wtts), "WtttA": f16(WtttA),
        "WtttB": f16(WtttB), "W2A": f16(W2A), "W2B": f16(W2B),
        "M2": f16(M2), "CrepC": f16(crepC), "CrepW": f16(crep_cw),
    }


WEIGHT_SHAPES = {
    "Wall": (89, NCOL), "Wtts": (25, H), "WtttA": (125, 128),
    "WtttB": (125, 32), "W2A": (128, 128), "W2B": (32, 32),
    "M2": (32, 32), "CrepC": (P, 125), "CrepW": (P, 32),
}

WS_M = 0
WS_G = 128


def _tile_body(ctx: ExitStack, tc: tile.TileContext, io, n_macro: int):
    nc = tc.nc
    s_d, t_d, stT_d, out_d, wd = io["s"], io["t"], io["stT"], io["out"], io["w"]

    const = ctx.enter_context(tc.tile_pool(name="const", bufs=1))
    W = {}
    for name, shp in WEIGHT_SHAPES.items():
        W[name] = const.tile(list(shp), F16, tag=name, name=f"W_{name}")
        nc.sync.dma_start(W[name][:], wd[name])
    ident32 = const.tile([128, 128], F32, tag="ident32")
    masks.make_identity(nc, ident32[:])

    io_pool = ctx.enter_context(tc.tile_pool(name="io", bufs=2))
    zq = ctx.enter_context(tc.tile_pool(name="zq", bufs=2))
    sb = ctx.enter_context(tc.tile_pool(name="sb", bufs=2))
    fm = ctx.enter_context(tc.tile_pool(name="fm", bufs=2))
    zps = ctx.enter_context(tc.tile_pool(name="zps", bufs=1, space="PSUM"))
    tpsA = ctx.enter_context(tc.tile_pool(name="tpsA", bufs=2, space="PSUM"))
    tpsB = ctx.enter_context(tc.tile_pool(name="tpsB", bufs=1, space="PSUM"))

    ctx.enter_context(nc.allow_low_precision("fp16 intermediates fit the 2e-2 budget"))

    for im in range(n_macro):
        r0 = im * MS
        st4 = io_pool.tile([P, G, 96], F16, tag="st4")
        nc.sync.dma_start(st4[:, :, 0:64], s_d[r0: r0 + MS, :].rearrange("(g p) u -> p g u", g=G))
        nc.sync.dma_start(st4[:, :, 64:89], t_d[r0: r0 + MS, :].rearrange("(g p) u -> p g u", g=G))
        stT4 = io_pool.tile([89, MS], F16, tag="stT4")
        nc.sync.dma_start(stT4[:], stT_d[:, r0: r0 + MS])

        zsqE = zq.tile([P, G, CE], F16, tag="zsqE")
        ae4 = sb.tile([P, G, 285], F16, tag="ae4")

        for g in range(G):
            lhs = stT4[:, g * P: (g + 1) * P]
            zp = zps.tile([P, NCOL], F32, tag="zp")
            for c0 in range(0, 2048, 512):
                nc.tensor.matmul(zp[:, c0: c0 + 512], lhs, W["Wall"][:, c0: c0 + 512], start=True, stop=True)
            nc.tensor.matmul(zp[:, 2048:NCOL], lhs, W["Wall"][:, 2048:NCOL], start=True, stop=True)
            # ACT squares everything (id + 32 eigen w's)
            nc.scalar.activation(zsqE[:, g, 0:1024], zp[:, 0:1024], AF.Square)
            nc.scalar.activation(zsqE[:, g, 1024:CE], zp[:, 1024:CE], AF.Square)
            nc.scalar.activation(ae4[:, g, :], zp[:, CE:NCOL], AF.Copy)

        # ---- h1 ----
        tmp_e = sb.tile([P, G, 33], F32, tag="tmp_e")
        zhE = zq.tile([P, G, 33, 32], F16, tag="zhE")
        zE4 = zsqE[:].rearrange("p g (w h v) -> p (g w) h v", h=2, v=32)
        nc.gpsimd.tensor_tensor(
            zhE[:].rearrange("p g w v -> p (g w) v"), zE4[:, :, 0, :], zE4[:, :, 1, :], OP.add)
        nc.vector.tensor_reduce(
            tmp_e[:], zhE[:].rearrange("p g w v -> p (g w) v"), axis=AX.X, op=OP.add)

        ws = sb.tile([P, G, 256], F16, tag="ws")
        h1f = sb.tile([P, G, 32], F32, tag="h1f")
        tmp2 = sb.tile([P, G, 32], F32, tag="tmp2")
        s2b = tmp_e[:, :, 0:1].to_broadcast((P, G, 32))
        cwb = W["CrepW"][:].unsqueeze(1).to_broadcast((P, G, 32))
        nc.vector.tensor_tensor(tmp2[:], s2b, cwb, OP.mult)
        nc.vector.tensor_tensor(h1f[:], tmp_e[:, :, 1:33], tmp2[:], OP.subtract)

        # ---- small bilinears ----
        tkr = sb.tile([P, G, 25], F16, tag="tkr")
        nc.gpsimd.tensor_copy(
            tkr[:].rearrange("p g (k r) -> p g k r", k=5),
            st4[:, :, 64:89].rearrange("p g (r k) -> p g k r", r=5))
        q24 = zq.tile([P, G, 800], F16, tag="q24")
        for g in range(G):
            a4v = ae4[:, g, 0:160].rearrange("p (w r) -> p w r", w=H).unsqueeze(1).to_broadcast((P, 5, H, 5))
            t_kr = tkr[:, g, :].rearrange("p (k r) -> p k r", k=5).unsqueeze(2).to_broadcast((P, 5, H, 5))
            eng = nc.gpsimd if g < 3 else nc.vector
            eng.tensor_tensor(q24[:, g, :].rearrange("p (k w r) -> p k w r", k=5, w=H), a4v, t_kr, OP.mult)
        ht1f = sb.tile([P, G, 160], F32, tag="ht1f")
        nc.vector.tensor_reduce(
            ht1f[:],
            q24[:].rearrange("p g (c r) -> p (g c) r", r=5), axis=AX.X, op=OP.add)

        qg4 = zq.tile([P, G, 125], F16, tag="qg4")
        for g in range(G):
            t_ui = st4[:, g, 64:89].rearrange("p (u i) -> p u i", u=5).unsqueeze(2).to_broadcast((P, 5, 5, 5))
            t_vi = st4[:, g, 64:89].rearrange("p (v i) -> p v i", v=5).unsqueeze(1).to_broadcast((P, 5, 5, 5))
            eng = nc.gpsimd if g < 3 else nc.vector
            eng.tensor_tensor(qg4[:, g, :].rearrange("p (u v i) -> p u v i", u=5, v=5), t_ui, t_vi, OP.mult)
        nc.vector.tensor_reduce(
            ws[:, :, WS_G: WS_G + 25],
            qg4[:].rearrange("p g (c i) -> p (g c) i", i=5), axis=AX.X, op=OP.add)

        q74 = zq.tile([P, G, 625], F16, tag="q74")
        for g in range(G):
            E3 = ae4[:, g, 160:285].rearrange("p (c j) -> p c j", j=5).unsqueeze(2).to_broadcast((P, 25, 5, 5))
            t_vj = st4[:, g, 64:89].rearrange("p (v j) -> p v j", v=5).unsqueeze(1).to_broadcast((P, 25, 5, 5))
            eng = nc.gpsimd if g < 3 else nc.vector
            eng.tensor_tensor(q74[:, g, :].rearrange("p (c v j) -> p c v j", c=25, v=5), E3, t_vj, OP.mult)
        nc.vector.tensor_reduce(
            ws[:, :, WS_M: WS_M + 125],
            q74[:].rearrange("p g (c j) -> p (g c) j", j=5), axis=AX.X, op=OP.add)

        # ---- XBAR forward ----
        xb = fm.tile([P, 2 * G, P], F16, tag="xb")
        nc.sync.dma_start_transpose(xb[:], ws[:].rearrange("p g c -> p (g c)"))
        mt_rhs = xb[0:125, :, :].rearrange("f (g b) s -> f g b s", b=2)[:, :, 0, :]
        gt_rhs = xb[0:25, :, :].rearrange("f (g b) s -> f g b s", b=2)[:, :, 1, :]

        # ---- tp2 feature-major; psum evicted by ACT, added by Pool ----
        htA_ps = tpsA.tile([P, MS], F32, tag="tpA")
        nc.tensor.matmul(htA_ps[:], W["WtttA"][:], mt_rhs, start=True, stop=False)
        for g in range(G):
            nc.tensor.matmul(htA_ps[:, g * P:(g + 1) * P], ht1f[:, g, 0:128], ident32[:],
                             is_transpose=True, start=False, stop=(g == G - 1))
        htB_ps = tpsB.tile([32, MS], F32, tag="tpB")
        nc.tensor.matmul(htB_ps[:], W["WtttB"][:], mt_rhs, start=True, stop=False)
        for g in range(G):
            nc.tensor.matmul(htB_ps[:, g * P:(g + 1) * P], ht1f[:, g, 128:160], ident32[:],
                             is_transpose=True, start=False, stop=(g == G - 1))
        htA = fm.tile([P, MS], F16, tag="htAf")
        nc.scalar.copy(htA[:], htA_ps[:])
        pk96 = fm.tile([96, MS], F16, tag="pk96")
        nc.scalar.copy(pk96[0:32, :], htB_ps[:])

        hs_ps = tpsB.tile([32, MS], F32, tag="tpB")
        nc.tensor.matmul(hs_ps[:], W["Wtts"][:], gt_rhs, start=True, stop=False)
        for g in range(G):
            nc.tensor.matmul(hs_ps[:, g * P:(g + 1) * P], h1f[:, g, :], ident32[:],
                             is_transpose=True, start=False, stop=(g == G - 1))
        hs_fm = fm.tile([32, MS], F16, tag="hsf")
        nc.scalar.copy(hs_fm[:], hs_ps[:])

        al_ps = tpsB.tile([32, MS], F32, tag="tpB")
        nc.tensor.matmul(al_ps[:], W["M2"][:], hs_fm[:], start=True, stop=True)
        nc.scalar.activation(pk96[64:96, :], al_ps[:], AF.Copy)

        g2A_ps = tpsA.tile([P, MS], F32, tag="tpA")
        nc.tensor.matmul(g2A_ps[:], W["W2A"][:], htA[:], start=True, stop=True)
        g2B_ps = tpsB.tile([32, MS], F32, tag="tpB")
        nc.tensor.matmul(g2B_ps[:], W["W2B"][:], pk96[0:32, :], start=True, stop=True)
        g2A = fm.tile([P, MS], F16, tag="g2Af")
        nc.scalar.activation(g2A[:], g2A_ps[:], AF.Copy)
        nc.scalar.activation(pk96[32:64, :], g2B_ps[:], AF.Copy)

        # ---- XBAR back ----
        xhA = fm.tile([P, G, P], F16, tag="xhA")
        nc.sync.dma_start_transpose(xhA[:], htA[:])
        xgA = fm.tile([P, G, P], F16, tag="xgA")
        nc.sync.dma_start_transpose(xgA[:], g2A[:])
        xp = fm.tile([P, G, 96], F16, tag="xp")
        nc.sync.dma_start_transpose(xp[:], pk96[:])

        htb4 = sb.tile([P, G, 160], F16, tag="htb4")
        nc.gpsimd.tensor_copy(htb4[:, :, 0:128], xhA[:])
        nc.gpsimd.tensor_copy(htb4[:, :, 128:160], xp[:, :, 0:32])
        g2b4 = sb.tile([P, G, 160], F16, tag="g2b4")
        nc.gpsimd.tensor_copy(g2b4[:, :, 0:128], xgA[:])
        nc.gpsimd.tensor_copy(g2b4[:, :, 128:160], xp[:, :, 32:64])

        qq4 = zq.tile([P, G, 800], F16, tag="qq4")
        for g in range(G):
            g2v = g2b4[:, g, :].rearrange("p (i v) -> p i v", i=5).unsqueeze(2).to_broadcast((P, 5, 5, 32))
            htv = htb4[:, g, :].rearrange("p (j v) -> p j v", j=5).unsqueeze(1).to_broadcast((P, 5, 5, 32))
            eng = nc.gpsimd if g < 2 else nc.vector
            eng.tensor_tensor(qq4[:, g, :].rearrange("p (i j v) -> p i j v", i=5, j=5), g2v, htv, OP.mult)
        Q4 = sb.tile([P, G, 25], F16, tag="Q4")
        qqh = zq.tile([P, G, 25, 16], F16, tag="qqh")
        qv = qq4[:].rearrange("p g (c v) -> p (g c) v", v=32)
        nc.gpsimd.tensor_tensor(
            qqh[:].rearrange("p g c v -> p (g c) v"), qv[:, :, 0:16], qv[:, :, 16:32], OP.add)
        nc.vector.tensor_reduce(
            Q4[:], qqh[:].rearrange("p g c v -> p (g c) v"), axis=AX.X, op=OP.add)

        q10 = zq.tile([P, G, 125], F16, tag="q10")
        for g in range(G):
            Qb = Q4[:, g, :].unsqueeze(1).to_broadcast((P, 5, 25))
            cv = W["CrepC"][:].rearrange("p (k c) -> p k c", k=5)
            eng = nc.gpsimd if g < 2 else nc.vector
            eng.tensor_tensor(q10[:, g, :].rearrange("p (k c) -> p k c", k=5), Qb, cv, OP.mult)
        q12 = zq.tile([P, G, 160], F16, tag="q12")
        for g in range(G):
            alb = xp[:, g, 64:96].unsqueeze(1).to_broadcast((P, 5, 32))
            htk = htb4[:, g, :].rearrange("p (k v) -> p k v", k=5)
            eng = nc.gpsimd if g < 2 else nc.vector
            eng.tensor_tensor(q12[:, g, :].rearrange("p (k v) -> p k v", k=5), alb, htk, OP.mult)
        o14 = sb.tile([P, G, 5], F32, tag="o14")
        nc.vector.tensor_reduce(
            o14[:], q10[:].rearrange("p g (c j) -> p (g c) j", j=25), axis=AX.X, op=OP.add)
        o24 = sb.tile([P, G, 5], F32, tag="o24")
        nc.vector.tensor_reduce(
            o24[:], q12[:].rearrange("p g (c v) -> p (g c) v", v=32), axis=AX.X, op=OP.add)
        out4 = io_pool.tile([P, G, 5], F32, tag="out4")
        nc.gpsimd.tensor_add(out4[:], o14[:], o24[:])
        nc.sync.dma_start(out_d[r0: r0 + MS, :].rearrange("(g p) c -> p g c", g=G), out4[:])


def build_program(n_macro=NMACRO):
    nc = bacc.Bacc("TRN2", target_bir_lowering=False, debug=False,
                   enable_asserts=False, num_devices=NCORES)
    rows = n_macro * MS
    io = {
        "s": nc.dram_tensor("s", [rows, NS], F16, kind="ExternalInput").ap(),
        "t": nc.dram_tensor("t", [rows, 25], F16, kind="ExternalInput").ap(),
        "stT": nc.dram_tensor("stT", [89, rows], F16, kind="ExternalInput").ap(),
        "out": nc.dram_tensor("out", [rows, 5], F32, kind="ExternalOutput").ap(),
        "w": {name: nc.dram_tensor(name, list(shp), F16, kind="ExternalInput").ap()
              for name, shp in WEIGHT_SHAPES.items()},
    }
    with tile.TileContext(nc) as tc:
        with ExitStack() as ctx:
            _tile_body(ctx, tc, io, n_macro)
    nc.compile()
    return nc


def make_in_maps(
    scalars, kernel_t2_sum, mc_t2, coulomb_t2, bs_t2, mopac_coulomb_t2,
    w1_sss, w1_stt, w1_tst, w1_tts, w1_ttt, w2_stt, w2_tst, w2_ttt,
):
    wmap = prep_weights(w1_sss, w1_stt, w1_tst, w1_tts, w1_ttt, w2_stt, w2_tst, w2_ttt)
    s = np.ascontiguousarray(np.asarray(scalars, np.float16))
    t = np.stack(
        [np.asarray(kernel_t2_sum, np.float32), np.asarray(mc_t2, np.float32),
         np.asarray(coulomb_t2, np.float32), np.asarray(bs_t2, np.float32),
         np.asarray(mopac_coulomb_t2, np.float32)], axis=1,
    ).reshape(B, 25).astype(np.float16)
    in_maps = []
    for c in range(NCORES):
        sh = s[c * BPC: (c + 1) * BPC]
        th = t[c * BPC: (c + 1) * BPC]
        stT = np.concatenate([sh.T, th.T], axis=0)
        m = {"s": sh, "t": np.ascontiguousarray(th), "stT": np.ascontiguousarray(stT)}
        m.update(wmap)
        in_maps.append(m)
    return in_maps


_CACHED_NC = None


def kernel(
    scalars, kernel_t2_sum, mc_t2, coulomb_t2, bs_t2, mopac_coulomb_t2,
    w1_sss, w1_stt, w1_tst, w1_tts, w1_ttt, w2_stt, w2_tst, w2_ttt,
):
    global _CACHED_NC
    if _CACHED_NC is None:
        _CACHED_NC = build_program(NMACRO)
    nc = _CACHED_NC
    in_maps = make_in_maps(
        scalars, kernel_t2_sum, mc_t2, coulomb_t2, bs_t2, mopac_coulomb_t2,
        w1_sss, w1_stt, w1_tst, w1_tts, w1_ttt, w2_stt, w2_tst, w2_ttt,
    )
    res = run_bass_kernel_spmd(nc, in_maps, list(range(NCORES)))
    out = np.concatenate([res.results[c]["out"] for c in range(NCORES)], axis=0)
    return out.astype(np.float32)
